# revision 6
# baseline (speedup 1.0000x reference)
"""Trainium2 Bass kernel for nn_ConvGraph_SC (gnn_message_passing).

Reference computation (per batch b of 64, N=32 nodes, C=512 channels, 7x7 spatial):
    state = input.mean(axis=(3,4))                       # [B, N, C]
    mat1  = state @ W1.T + b1                            # [B, N, C]
    mat2  = state @ W2.T + b2                            # [B, N, C]
    adj   = mat1 @ mat2.T                                # [B, N, N]
    soft  = softmax((adj - mean(adj)) / std(adj), rows)  # global mean/std, ddof=1
    out   = mean(soft @ state + state, axis=1)           # [B, C]

Key algebraic simplifications used on-device:
  * Row softmax is invariant to the global mean shift -> only 1/std matters.
  * out[b,c] = (1/N) * sum_m (colsum(soft)[m] + 1) * state[m,c]
    -> no need to materialize new_state; a single weighted column-sum suffices.
  * The 1/49 spatial-mean scale is folded into host-side pre-scaled weights,
    and 1/(N*49) into the final weight vector.

Sharding: pure data parallel, 8 batches per NeuronCore, weights replicated.
"""

import numpy as np

import concourse.bacc as bacc
import concourse.tile as tile
from concourse import masks, mybir
from concourse.bass_utils import run_bass_kernel_spmd

F32 = mybir.dt.float32
NCORES = 8
B, N, C, HW = 64, 32, 512, 49
BPC = B // NCORES          # batches per core
FLAT = N * C * HW          # 802816 floats per batch
FREE = FLAT // 128         # 6272 floats per partition per batch
HALF = FREE // 2           # 3136

_CACHED_NC = None


def build_bass():
    nc = bacc.Bacc("TRN2", target_bir_lowering=False)

    x_d = nc.declare_dram_parameter("x", [BPC, 128, FREE], F32, isOutput=False)
    w1_d = nc.declare_dram_parameter("w1t", [C, C], F32, isOutput=False)
    w2_d = nc.declare_dram_parameter("w2t", [C, C], F32, isOutput=False)
    b1_d = nc.declare_dram_parameter("b1bc", [128, 128], F32, isOutput=False)
    b2_d = nc.declare_dram_parameter("b2bc", [128, 128], F32, isOutput=False)
    out_d = nc.declare_dram_parameter("out", [128, 4 * BPC], F32, isOutput=True)

    with tile.TileContext(nc) as tc:
        with (
            tc.tile_pool(name="xpool", bufs=4) as xpool,
            tc.tile_pool(name="singles", bufs=1) as singles,
            tc.tile_pool(name="srawp", bufs=2) as srawp,
            tc.tile_pool(name="work", bufs=3) as work,
            tc.tile_pool(name="small", bufs=4) as small,
            tc.tile_pool(name="ps_t", bufs=1, space="PSUM") as ps_t_pool,
            tc.tile_pool(name="ps_mm", bufs=2, space="PSUM") as ps_mm_pool,
            tc.tile_pool(name="ps_adj", bufs=2, space="PSUM") as ps_adj_pool,
            tc.tile_pool(name="ps_misc", bufs=2, space="PSUM") as ps_misc_pool,
        ):
            # ---- one-time setup -------------------------------------------
            ident = singles.tile([128, 128], F32)
            masks.make_identity(nc, ident[:])

            ones_col = singles.tile([32, 1], F32)
            nc.vector.memset(ones_col[:], 1.0)
            ones_row = singles.tile([1, 128], F32)
            nc.vector.memset(ones_row[:], 1.0)

            # weights: block r (c in [128r,128r+128)) lives at cols [512r, 512r+512)
            w1sb = singles.tile([128, 4 * C], F32)
            w2sb = singles.tile([128, 4 * C], F32)
            for r in range(4):
                nc.sync.dma_start(
                    out=w1sb[:, 512 * r : 512 * (r + 1)],
                    in_=w1_d[128 * r : 128 * (r + 1), :],
                )
                nc.sync.dma_start(
                    out=w2sb[:, 512 * r : 512 * (r + 1)],
                    in_=w2_d[128 * r : 128 * (r + 1), :],
                )
            b1sb = singles.tile([128, 128], F32)
            b2sb = singles.tile([128, 128], F32)
            nc.sync.dma_start(out=b1sb[:], in_=b1_d[:])
            nc.sync.dma_start(out=b2sb[:], in_=b2_d[:])

            outsb = singles.tile([128, 4 * BPC], F32)

            # ---- per-batch pipeline ---------------------------------------
            for b in range(BPC):
                # spatial sum: x[b] flat [128, 6272] -> sraw [128, 128]
                # partition p = 4k + r holds channels [128r,128r+128) of node k,
                # 49 spatial elements each -> sraw[4k+r, j] = sum_s x[b,k,128r+j,s]
                sraw = srawp.tile([128, 128], F32)
                for h in range(2):
                    xb = xpool.tile([128, HALF], F32, tag="xb")
                    nc.sync.dma_start(
                        out=xb[:], in_=x_d[b, :, HALF * h : HALF * (h + 1)]
                    )
                    nc.vector.reduce_sum(
                        out=sraw[:, 64 * h : 64 * (h + 1)],
                        in_=xb[:].rearrange("p (g s) -> p g s", s=HW),
                        axis=mybir.AxisListType.X,
                    )

                # transpose: st_t[j, 4k+r] = sraw[4k+r, j]
                ps_t = ps_t_pool.tile([128, 128], F32)
                nc.tensor.transpose(ps_t[:], sraw[:], ident[:])
                st_t = work.tile([128, 128], F32)
                nc.vector.tensor_copy(st_t[:], ps_t[:])

                # mat1T/mat2T: psum col 32s+k = mat{1,2}T[d=128s+d', n=k] (no bias)
                ps_mm = ps_mm_pool.tile([128, 256], F32)
                for s in range(4):
                    for r in range(4):
                        nc.tensor.matmul(
                            ps_mm[:, 32 * s : 32 * (s + 1)],
                            w1sb[:, 512 * r + 128 * s : 512 * r + 128 * (s + 1)],
                            st_t[:, r::4],
                            start=(r == 0), stop=(r == 3),
                        )
                    for r in range(4):
                        nc.tensor.matmul(
                            ps_mm[:, 128 + 32 * s : 128 + 32 * (s + 1)],
                            w2sb[:, 512 * r + 128 * s : 512 * r + 128 * (s + 1)],
                            st_t[:, r::4],
                            start=(r == 0), stop=(r == 3),
                        )

                m1 = work.tile([128, 128], F32, tag="m1")
                m2 = work.tile([128, 128], F32, tag="m2")
                nc.vector.tensor_add(m1[:], ps_mm[:, 0:128], b1sb[:])
                nc.vector.tensor_add(m2[:], ps_mm[:, 128:256], b2sb[:])

                # adj[n, m] = sum_d mat1T[d, n] * mat2T[d, m]
                ps_adj = ps_adj_pool.tile([32, 32], F32)
                for s in range(4):
                    nc.tensor.matmul(
                        ps_adj[:],
                        m1[:, 32 * s : 32 * (s + 1)],
                        m2[:, 32 * s : 32 * (s + 1)],
                        start=(s == 0), stop=(s == 3),
                    )

                # global stats: S1 = sum(adj), S2 = sum(adj^2) over all 1024
                stats = small.tile([32, 2], F32, tag="stats")
                nc.vector.reduce_sum(
                    out=stats[:, 0:1], in_=ps_adj[:], axis=mybir.AxisListType.X
                )
                sq = small.tile([32, 32], F32, tag="sq")
                nc.scalar.activation(
                    out=sq[:], in_=ps_adj[:],
                    func=mybir.ActivationFunctionType.Square,
                    accum_out=stats[:, 1:2],
                )

                ps_misc = ps_misc_pool.tile([128, 128], F32)
                # cross-partition sum -> [1, 2] {S1, S2}
                nc.tensor.matmul(
                    ps_misc[:1, 0:2], ones_col[:], stats[:], start=True, stop=True
                )
                s_sb = small.tile([1, 2], F32, tag="s_sb")
                nc.vector.tensor_copy(s_sb[:], ps_misc[:1, 0:2])
                # broadcast back to 32 partitions
                nc.tensor.matmul(
                    ps_misc[:32, 4:6], ones_row[:1, 0:32], s_sb[:],
                    start=True, stop=True,
                )
                s_all = small.tile([32, 2], F32, tag="s_all")
                nc.vector.tensor_copy(s_all[:], ps_misc[:32, 4:6])

                # inv_std = 1/sqrt((S2 - S1^2/1024)/1023)  (ddof=1; mean shift
                # cancels inside the row softmax so it is dropped entirely)
                t1 = small.tile([32, 1], F32, tag="t1")
                nc.vector.tensor_mul(t1[:], s_all[:, 0:1], s_all[:, 0:1])
                v1023 = small.tile([32, 1], F32, tag="v1023")
                nc.vector.tensor_scalar(
                    out=v1023[:], in0=t1[:],
                    scalar1=-1.0 / 1024.0, scalar2=s_all[:, 1:2],
                    op0=mybir.AluOpType.mult, op1=mybir.AluOpType.add,
                )
                stdt = small.tile([32, 1], F32, tag="stdt")
                nc.scalar.activation(
                    out=stdt[:], in_=v1023[:],
                    func=mybir.ActivationFunctionType.Sqrt,
                    scale=1.0 / 1023.0,
                )
                inv = small.tile([32, 1], F32, tag="inv")
                nc.vector.reciprocal(inv[:], stdt[:])

                # row softmax of adj * inv_std, fused: exp(adj*inv - rowmax*inv)
                negmax = small.tile([32, 1], F32, tag="negmax")
                nc.vector.reduce_max(
                    out=negmax[:], in_=ps_adj[:], axis=mybir.AxisListType.X,
                    negate=True,
                )
                negm = small.tile([32, 1], F32, tag="negm")
                nc.vector.tensor_mul(negm[:], negmax[:], inv[:])
                expt = small.tile([32, 32], F32, tag="expt")
                rowsum = small.tile([32, 1], F32, tag="rowsum")
                nc.scalar.activation(
                    out=expt[:], in_=ps_adj[:],
                    func=mybir.ActivationFunctionType.Exp,
                    bias=negm[:], scale=inv[:], accum_out=rowsum[:],
                )
                recip = small.tile([32, 1], F32, tag="recip")
                nc.vector.reciprocal(recip[:], rowsum[:])

                # w[m] = colsum(soft)[m] = sum_n recip[n]*expt[n,m]; then
                # wf = (w + 1) / (N*49) folds residual + node-mean + spatial-mean
                nc.tensor.matmul(
                    ps_misc[:1, 8:40], recip[:], expt[:], start=True, stop=True
                )
                w_sb = small.tile([1, 32], F32, tag="w_sb")
                nc.scalar.activation(
                    out=w_sb[:], in_=ps_misc[:1, 8:40],
                    func=mybir.ActivationFunctionType.Copy,
                    bias=1.0 / (N * HW), scale=1.0 / (N * HW),
                )
                # broadcast wf to 128 partitions
                nc.tensor.matmul(
                    ps_misc[:, 64:96], ones_row[:], w_sb[:], start=True, stop=True
                )

                # out[c=128r+j] = sum_k st_t[j, 4k+r] * wf[k]
                wb_sb = small.tile([128, 32], F32, tag="wb_sb")
                nc.vector.tensor_copy(wb_sb[:], ps_misc[:, 64:96])
                for r in range(4):
                    scr = small.tile([128, 32], F32, tag="scr")
                    nc.vector.tensor_mul(scr[:], st_t[:, r::4], wb_sb[:])
                    nc.vector.reduce_sum(
                        out=outsb[:, 4 * b + r : 4 * b + r + 1],
                        in_=scr[:],
                        axis=mybir.AxisListType.X,
                    )

            nc.sync.dma_start(out=out_d[:], in_=outsb[:])

    nc.finalize()
    return nc


def kernel(input, W1, b1, W2, b2):
    global _CACHED_NC
    if _CACHED_NC is None:
        _CACHED_NC = build_bass()
    nc = _CACHED_NC

    input = np.ascontiguousarray(input, dtype=np.float32)
    w1t = np.ascontiguousarray(W1.T.astype(np.float32) / np.float32(HW))
    w2t = np.ascontiguousarray(W2.T.astype(np.float32) / np.float32(HW))
    # b{1,2}bc[p, 32s+k] = b[128s+p]
    b1bc = np.ascontiguousarray(
        np.repeat(b1.astype(np.float32).reshape(4, 128).T, 32, axis=1)
    )
    b2bc = np.ascontiguousarray(
        np.repeat(b2.astype(np.float32).reshape(4, 128).T, 32, axis=1)
    )

    in_maps = []
    for i in range(NCORES):
        shard = input[BPC * i : BPC * (i + 1)].reshape(BPC, 128, FREE)
        in_maps.append(
            {"x": shard, "w1t": w1t, "w2t": w2t, "b1bc": b1bc, "b2bc": b2bc}
        )

    res = run_bass_kernel_spmd(nc, in_maps, list(range(NCORES)))

    out = np.empty((B, C), dtype=np.float32)
    for i in range(NCORES):
        o = res.results[i]["out"]  # [128, 4*BPC], col = 4b + r
        out[BPC * i : BPC * (i + 1)] = (
            o.reshape(128, BPC, 4).transpose(1, 2, 0).reshape(BPC, C)
        )
    return out
